# revision 39
# baseline (speedup 1.0000x reference)
"""Trainium2 Bass kernel for nn_ECA (attention block + residual + LayerNorm).

Reference computation (per batch b):
    qkv = x @ qkv_w.T ; q,k,v per head
    attn = softmax((q @ k.T) * sqrt(D))
    x1 = attn @ v  -> concat heads -> @ proj_w.T + proj_b
    out = LayerNorm(x + x1) * gamma + beta     # eps 1e-5

Sharding: 8 cores = 4 batches x 2 query-halves. Each core receives the full
batch's tokens ("xh16", rolled so its own 1024 query tokens are rows 0:1024),
computes K/V for all 2048 keys (duplicated across the 2 cores of a batch),
attention + proj + LN for its 1024 queries. No collectives.

v3: single-limb fp16 throughout (host numpy sim of the full fp16 pipeline:
rel err 3.7e-3 vs the 2e-2 gate). Q^T/K^T live in natural head-pair layout
qg/kg[g] [128 = dims of heads 2g,2g+1, N]; S is one K=64 matmul per 512-key
chunk (lhsT and rhs both at partition base r = (h*64)%128).

v4 scheduling: Phase A is split -- heads 0-5's K/Q/V are computed up front
(weights DMA'd on the ACT hwdge queue in parallel with the x^T transposes on
sync), heads 6-11's K/Q/V are deferred and injected one group per tile-slot
into qb0's attention loop, riding the pb_s psum rotation (third slot of the
q0/q1/def cycle). qb0's proj+LN is likewise injected into qb1's early head
slots (pp chunks ride the same rotation). The softmax normalize alternates
between DVE (tensor_scalar) and ACT (activation Identity, scale=e*rl) to
balance the two engines; AV psum->x1t copies go to ACT for even heads
(partition base 0) and DVE for odd heads (base 64 -- ACT cannot shift
partition base). Known HW landmines: tensor_tensor_reduce hangs the device,
ACT copies cannot shift partition base (DVE can), gpsimd tensor ops on tiny
tiles fault, SBUF<->SBUF DMAs deadlock against xbar transposes.
"""

import sys
from dataclasses import dataclass

import numpy as np

try:
    import concourse.bass  # noqa: F401
except ImportError:  # fresh dir without sitecustomize path
    sys.path.insert(0, "/opt/trn_rl_repo")


@dataclass(frozen=True)
class Cfg:
    Nk: int = 2048   # keys per core (full batch)
    Nq: int = 1024   # queries per core
    C: int = 768     # model dim (also total head dim H*D)
    H: int = 12
    D: int = 64
    lowp: str | None = None  # unused; kept for test harness compat

    @property
    def CH(self):
        return self.C // 128

    @property
    def G(self):
        return (self.H * self.D) // 128

    @property
    def TQ(self):
        return self.Nq // 128

    @property
    def TK(self):
        return self.Nk // 128

    @property
    def slabs(self):
        return self.Nk // self.Nq


def build_program(cfg: Cfg):
    from collections import deque

    import concourse.bass as bass
    import concourse.mybir as mybir
    import concourse.tile as tile

    from concourse import bacc

    f32 = mybir.dt.float32
    f16 = mybir.dt.float16
    ts = bass.ts
    Nk, Nq, C, H, D = cfg.Nk, cfg.Nq, cfg.C, cfg.H, cfg.D
    CH, G, TQ, TK = cfg.CH, cfg.G, cfg.TQ, cfg.TK
    QC = H * D
    assert QC % 128 == 0 and C % 128 == 0 and Nq % 128 == 0

    nc = bacc.Bacc("TRN2", target_bir_lowering=False, debug=False, num_devices=8)

    xh_d = nc.dram_tensor("xh16", [Nk, C], f16, kind="ExternalInput")
    xq_d = nc.dram_tensor("xq", [Nq, C], f32, kind="ExternalInput")  # x + proj_b
    wq_d = nc.dram_tensor("wq_h", [C, QC], f16, kind="ExternalInput")
    wk_d = nc.dram_tensor("wk_h", [C, QC], f16, kind="ExternalInput")
    wv_d = nc.dram_tensor("wv_h", [C, QC], f16, kind="ExternalInput")
    wp_d = nc.dram_tensor("wp_h", [QC, C], f16, kind="ExternalInput")
    vec_d = nc.dram_tensor("vecs", [2, C], f32, kind="ExternalInput")  # gamma, beta
    out_d = nc.dram_tensor("out", [Nq, C], f32, kind="ExternalOutput")

    J = 512              # matmul free-dim chunk (one psum bank)
    JQ = 1024            # softmax quarter width (2 psum banks)
    NQS = Nk // JQ       # quarters per row (2)
    BLK = min(4, TQ)     # q-tiles per AV block
    GLO = G // 2         # head-pair groups computed up front (g 0..2)

    with tile.TileContext(nc) as tc:
        with tc.tile_pool(name="persist", bufs=1) as persist:
            # kg[g]/qg[g]: K^T/Q^T in natural layout, rows = dims of heads
            # 2g (0:64) and 2g+1 (64:128).
            kg = [persist.tile([128, Nk], f16, name=f"kg{g}", tag=f"kg{g}")
                  for g in range(G)]
            qg = [persist.tile([128, Nq], f16, name=f"qg{g}", tag=f"qg{g}")
                  for g in range(G)]
            vb = [persist.tile([128, H, D], f16, name=f"vb{t}", tag=f"vb{t}") for t in range(TK)]
            # x^T persists into Phase B for the deferred K/Q/V groups
            xh2 = [persist.tile([128, CH, Nq], f16, name=f"xh{s}", tag=f"xh{s}")
                   for s in range(cfg.slabs)]

            def emit_kq_group(ps_pool, g, which, wgh, slab, j, on_dve=False):
                # one j-chunk of K^T (or Q^T): 6 matmuls + a cast-copy
                xh = xh2[slab]
                ps = ps_pool.tile([128, J], f32, name="ps_qk", tag=ps_pool._v4tag)
                for c in range(CH):
                    nc.tensor.matmul(ps[:], wgh[:, c, :], xh[:, c, ts(j, J)],
                                     start=(c == 0), stop=(c == CH - 1))
                cp = nc.vector.tensor_copy if on_dve else nc.scalar.copy
                if which == "k":
                    sl = slice(slab * Nq + j * J, slab * Nq + (j + 1) * J)
                    cp(kg[g][:, sl], ps[:])
                else:
                    cp(qg[g][:, ts(j, J)], ps[:])

            def emit_v_group(ps_pool, wvg, vc_base, slab, t, pw, on_dve=False):
                # V for one token tile: 6 matmuls + cast-copy (token-major)
                xh = xh2[slab]
                vw = 384
                psv = ps_pool.tile([128, pw], f32, name="psv", tag=ps_pool._v4tag)
                for c in range(CH):
                    nc.tensor.matmul(psv[:, :vw], xh[:, c, ts(t, 128)],
                                     wvg[:, c, :vw],
                                     start=(c == 0), stop=(c == CH - 1))
                hb = vc_base // D
                cp = nc.vector.tensor_copy if on_dve else nc.scalar.copy
                cp(vb[slab * TQ + t][:, hb:hb + vw // D, 0:D], psv[:, :vw])

            # ---------------- Phase A (inline half): x^T, g 0..2, V lo ----------------
            with tc.tile_pool(name="pa_w", bufs=6) as pa_w, \
                 tc.tile_pool(name="pa_wv", bufs=1) as pa_wv, \
                 tc.tile_pool(name="pa_ps", bufs=4, space="PSUM") as pa_ps, \
                 tc.tile_pool(name="pa_psv", bufs=4, space="PSUM") as pa_psv:
                pa_ps._v4tag = "ps_qk"
                pa_psv._v4tag = "psv"

                # weight DMAs go on sync BEFORE the x^T transposes (the ACT
                # queue is blocked by activation-table loads at startup)
                wlo = {}
                for g in range(GLO):
                    for which, w_d in (("k", wk_d), ("q", wq_d)):
                        wgh = pa_w.tile([128, CH, 128], f16, name="wgh", tag=f"w{which}{g}", bufs=1)
                        nc.sync.dma_start(wgh[:], w_d.ap()[:, ts(g, 128)].rearrange("(c p) n -> p c n", p=128))
                        wlo[(which, g)] = wgh
                wv_lo = pa_wv.tile([128, CH, 384], f16, name="wv_lo", tag="wv_lo")
                nc.scalar.dma_start(wv_lo[:], wv_d.ap()[:, 0:384].rearrange("(c p) n -> p c n", p=128))
                for slab in range(cfg.slabs):
                    for t in range(TQ):
                        row = slice((slab * TQ + t) * 128, (slab * TQ + t + 1) * 128)
                        nc.sync.dma_start(xh2[slab][:, :, ts(t, 128)], xh_d.ap()[row, :], transpose=True)

                # all slab-0 compute first (slab-1 transposes still landing)
                for g in range(GLO):
                    for j in range(Nq // J):
                        emit_kq_group(pa_ps, g, "k", wlo[("k", g)], 0, j)
                    for j in range(Nq // J):
                        emit_kq_group(pa_ps, g, "q", wlo[("q", g)], 0, j)
                for t in range(TQ):
                    emit_v_group(pa_psv, wv_lo, 0, 0, t, 384)
                for g in range(GLO):
                    for j in range(Nq // J):
                        emit_kq_group(pa_ps, g, "k", wlo[("k", g)], 1, j)
                for t in range(TQ):
                    emit_v_group(pa_psv, wv_lo, 0, 1, t, 384)

            # ---------------- Phase B: attention + proj + LN ----------------
            with tc.tile_pool(name="pc_w", bufs=1) as pc_w, \
                 tc.tile_pool(name="pd_w", bufs=4) as pd_w, \
                 tc.tile_pool(name="pb_p", bufs=5) as pb_p, \
                 tc.tile_pool(name="pb_pth", bufs=2) as pb_pth, \
                 tc.tile_pool(name="pb_st", bufs=6) as pb_st, \
                 tc.tile_pool(name="pc_sb", bufs=2) as pc_sb, \
                 tc.tile_pool(name="pc_st", bufs=3) as pc_st, \
                 tc.tile_pool(name="pb_s", bufs=3, space="PSUM") as pb_s, \
                 tc.tile_pool(name="pb_x1", bufs=2, space="PSUM") as pb_x1:
                pb_s._v4tag = "ps_s"
                x1t = [pc_w.tile([128, Nq], f16, name=f"x1t{g}", tag=f"x1t{g}")
                       for g in range(G)]

                # LN/proj prep
                ones = pc_w.tile([1, 128], f32, name="ones", tag="ones")
                nc.gpsimd.memset(ones[:], 1.0)
                bc = []
                for vi in range(2):
                    vrow = pc_w.tile([1, C], f32, name=f"vrow{vi}", tag=f"vrow{vi}")
                    nc.sync.dma_start(vrow[:], vec_d.ap()[vi:vi + 1, :])
                    bct = pc_w.tile([128, C], f32, name=f"bc{vi}", tag=f"bc{vi}")
                    for j in range(0, C, 512):
                        w = min(512, C - j)
                        psb = pb_x1.tile([128, 512], f32, name="psb", tag="ps_x1")
                        nc.tensor.matmul(psb[:, :w], ones[:], vrow[:, j:j + w],
                                         start=True, stop=True)
                        nc.scalar.copy(bct[:, j:j + w], psb[:, :w])
                    bc.append(bct)
                gam_bc, bet_bc = bc
                wpb = []
                for c in range(G):
                    wpc = pc_w.tile([128, C], f16, name=f"wpb{c}", tag=f"wpb{c}")
                    nc.sync.dma_start(wpc[:], wp_d.ap()[ts(c, 128), :])
                    wpb.append(wpc)
                eps_t = pc_w.tile([128, 1], f32, name="eps_t", tag="eps_t")
                nc.gpsimd.memset(eps_t[:], 1e-5)

                # -------- deferred Phase A (g 3..5 K/Q + V hi), injected --------
                wdef = {}

                def d_dma_w(which, g):
                    def run():
                        w_d = wk_d if which == "k" else wq_d
                        wgh = pd_w.tile([128, CH, 128], f16, name="wgh_d", tag="wd", bufs=2)
                        nc.gpsimd.dma_start(wgh[:], w_d.ap()[:, ts(g, 128)].rearrange("(c p) n -> p c n", p=128))
                        wdef[(which, g)] = wgh
                    return run, False

                def d_dma_wv():
                    def run():
                        wvg = pd_w.tile([128, CH, 384], f16, name="wv_hi", tag="wv_hi", bufs=1)
                        nc.gpsimd.dma_start(wvg[:], wv_d.ap()[:, 384:768].rearrange("(c p) n -> p c n", p=128))
                        wdef["v"] = wvg
                    return run, False

                dctr = [0]

                def d_kq(which, g, slab):
                    def run():
                        for j in range(Nq // J):
                            dctr[0] += 1
                            emit_kq_group(pb_s, g, which, wdef[(which, g)], slab, j,
                                          on_dve=(dctr[0] % 2 == 0))
                    return run, True

                def d_v(slab, t):
                    def run():
                        dctr[0] += 1
                        emit_v_group(pb_s, wdef["v"], 384, slab, t, J,
                                     on_dve=(dctr[0] % 2 == 0))
                    return run, True

                deferred = deque()
                for g in range(GLO, G):
                    deferred.append(d_dma_w("k", g))
                    deferred.append(d_kq("k", g, 0))
                    deferred.append(d_kq("k", g, 1))
                    deferred.append(d_dma_w("q", g))
                    deferred.append(d_kq("q", g, 0))
                    if g == GLO:
                        deferred.append(d_dma_wv())
                    for t in range(2):
                        deferred.append(d_v(0, (g - GLO) * 2 + t))
                for t in range(2 * (G - GLO), TQ):
                    deferred.append(d_v(0, t))
                for t in range(TQ):
                    deferred.append(d_v(1, t))

                def inject_deferred():
                    # run dma thunks for free; one compute group per call
                    while deferred:
                        run, is_compute = deferred.popleft()
                        run()
                        if is_compute:
                            break

                KCH = TK // BLK  # AV k-chunks per tile slot

                def emit_av_chunk(st, kc):
                    # one quarter of the AV accumulation; spread one chunk per
                    # tile slot, starting a slot AFTER the head boundary so the
                    # last tile's normalize+transposes have landed by then
                    g, r, h, qb, pThb, holder = st
                    if kc == 0:
                        holder.append(pb_x1.tile([D, BLK * 128], f32, name="ps_x1", tag="ps_x1"))
                    ps_x1 = holder[0]
                    for k in range(kc * KCH, (kc + 1) * KCH):
                        nc.tensor.matmul(ps_x1[:],
                                         vb[k][:, h, 0:D],
                                         pThb[:, k, :, :].rearrange("p t q -> p (t q)"),
                                         start=(k == 0), stop=(k == TK - 1))
                    if kc == BLK - 1:
                        dst = x1t[g][r:r + D, qb * BLK * 128:(qb + 1) * BLK * 128]
                        if r == 0:
                            nc.scalar.copy(dst, ps_x1[:])  # no partition shift
                        else:
                            nc.vector.tensor_copy(dst, ps_x1[:])

                av_queue = deque()  # entries: [st, next_kc]

                def av_step():
                    if av_queue:
                        ent = av_queue[0]
                        emit_av_chunk(ent[0], ent[1])
                        ent[1] += 1
                        if ent[1] == BLK:
                            av_queue.popleft()

                KQ = JQ // 128

                def softmax_tail(p_t, e_p, l_pack, pThb, tt, on_act):
                    # l = sum_j l_j e_j; p_n = p_j * e_j / l. Issued one tile
                    # LATE so these small ops never block the next tile's big
                    # maxes/exps at the FIFO heads.
                    lw = pb_st.tile([128, NQS], f32, name="lw", tag="lw")
                    nc.vector.tensor_mul(lw[:], l_pack[:], e_p[:])
                    l_tot = pb_st.tile([128, 1], f32, name="l_tot", tag="l_tot")
                    nc.vector.reduce_sum(out=l_tot[:], in_=lw[:],
                                         axis=mybir.AxisListType.X)
                    rl = pb_st.tile([128, 1], f32, name="rl", tag="rl")
                    nc.vector.reciprocal(rl[:], l_tot[:])
                    p_n = pb_p.tile([128, Nk], f16, name="p_n", tag="p_t")
                    if on_act:
                        er = pb_st.tile([128, NQS], f32, name="er", tag="er")
                        nc.vector.tensor_scalar(out=er[:], in0=e_p[:], scalar1=rl[:],
                                                scalar2=None,
                                                op0=mybir.AluOpType.mult)
                    for j2 in range(NQS):
                        if on_act:
                            nc.scalar.activation(p_n[:, ts(j2, JQ)], p_t[:, ts(j2, JQ)],
                                                 mybir.ActivationFunctionType.Identity,
                                                 scale=er[:, j2:j2 + 1])
                        else:
                            nc.vector.tensor_scalar(
                                out=p_n[:, ts(j2, JQ)], in0=p_t[:, ts(j2, JQ)],
                                scalar1=e_p[:, j2:j2 + 1], scalar2=rl[:],
                                op0=mybir.AluOpType.mult, op1=mybir.AluOpType.mult)
                        # per-quarter transpose: lower latency, same bytes
                        nc.sync.dma_start(pThb[:, j2 * KQ:(j2 + 1) * KQ, tt, :],
                                          p_n[:, ts(j2, JQ)], transpose=True)

                # ---- proj + residual + LayerNorm, per tile, injectable ----
                NSTAT = 256
                nsub = C // NSTAT
                xr_t = {}

                def proj_prefetch(t):
                    xr = pc_sb.tile([128, C], f16, name="xr", tag="xrh", bufs=4)
                    nc.gpsimd.dma_start(xr[:], xq_d.ap()[ts(t, 128), :])
                    xr_t[t] = xr

                def proj_chunk(t, j, w, pool, tag):
                    pp = pool.tile([128, w], f32, name="pp", tag=tag)
                    for c in range(G):
                        nc.tensor.matmul(pp[:, :w], x1t[c][:, ts(t, 128)], wpb[c][:, j:j + w],
                                         start=(c == 0), stop=(c == G - 1))
                    return (j, w, pp)

                def proj_finish(t, pps, affine_dve=False):
                    xr = xr_t.pop(t)
                    u = pc_sb.tile([128, C], f32, name="u", tag="u")
                    for (j, w, pp) in pps:
                        nc.vector.tensor_add(u[:, j:j + w], pp[:, :w], xr[:, j:j + w])
                    stats = pc_st.tile([128, nsub, 6], f32, name="stats", tag="stats")
                    for s in range(nsub):
                        nc.vector.bn_stats(out=stats[:, s, :], in_=u[:, ts(s, NSTAT)])
                    mv = pc_st.tile([128, 2], f32, name="mv", tag="mv")
                    nc.vector.bn_aggr(out=mv[:], in_=stats[:])
                    rstd = pc_st.tile([128, 1], f32, name="rstd", tag="rstd")
                    nc.scalar.activation(rstd[:], mv[:, 1:2],
                                         mybir.ActivationFunctionType.Sqrt, bias=eps_t[:])
                    nc.vector.reciprocal(rstd[:], rstd[:])
                    nmr = pc_st.tile([128, 1], f32, name="nmr", tag="nmr")
                    nc.vector.tensor_scalar(out=nmr[:], in0=mv[:, 0:1],
                                            scalar1=rstd[:], scalar2=-1.0,
                                            op0=mybir.AluOpType.mult,
                                            op1=mybir.AluOpType.mult)
                    of = pc_sb.tile([128, C], f32, name="of", tag="u")
                    # (u - mu)*rstd on ACT, then *gamma, +beta on GpSimd
                    nc.scalar.activation(of[:], u[:],
                                         mybir.ActivationFunctionType.Identity,
                                         scale=rstd[:], bias=nmr[:])
                    if affine_dve:
                        # final-flush tiles: gpsimd chains would serialize the
                        # tail; DVE is idle there
                        nc.vector.tensor_mul(of[:], of[:], gam_bc[:])
                        nc.vector.tensor_add(of[:], of[:], bet_bc[:])
                    else:
                        nc.gpsimd.tensor_mul(of[:], of[:], gam_bc[:])
                        nc.gpsimd.tensor_add(of[:], of[:], bet_bc[:])
                    nc.sync.dma_start(out_d.ap()[ts(t, 128), :], of[:])

                def proj_full(t):
                    proj_prefetch(t)
                    pps = [proj_chunk(t, 0, 512, pb_x1, "ps_x1"),
                           proj_chunk(t, 512, 256, pb_x1, "ps_x1")]
                    proj_finish(t, pps)

                pending_sm = None
                slot = 0
                proj_state = {}  # t -> list of pps
                for qb in range(TQ // BLK):
                    for h in range(H):
                        g, r = divmod(h * D, 128)
                        for tt in range(BLK):
                            t = qb * BLK + tt
                            q_s = qg[g][r:r + D, ts(t, 128)]
                            p_t = pb_p.tile([128, Nk], f16, name="p_t", tag="p_t")
                            nm_pack = pb_st.tile([128, NQS], f32, name="nm_pack", tag="nm_pack")
                            l_pack = pb_st.tile([128, NQS], f32, name="l_pack", tag="l_pack")
                            for j2 in range(NQS):
                                ps_s = pb_s.tile([128, JQ], f32, name="ps_s", tag="ps_s")
                                for jj in range(JQ // J):
                                    sl = slice(j2 * JQ + jj * J, j2 * JQ + (jj + 1) * J)
                                    nc.tensor.matmul(ps_s[:, ts(jj, J)], q_s,
                                                     kg[g][r:r + D, sl],
                                                     start=True, stop=True)
                                nc.vector.reduce_max(out=nm_pack[:, j2:j2 + 1], in_=ps_s[:],
                                                     axis=mybir.AxisListType.X, negate=True)
                                nc.scalar.activation(p_t[:, ts(j2, JQ)], ps_s[:],
                                                     mybir.ActivationFunctionType.Exp,
                                                     bias=nm_pack[:, j2:j2 + 1],
                                                     accum_out=l_pack[:, j2:j2 + 1])
                            if tt == 0:
                                pThb = pb_pth.tile([128, TK, BLK, 128], f16, name="pThb", tag="pThb")
                            # negm/e_p stay in-tile (depend only on this tile's
                            # maxes); the lagged tail runs next tile
                            negm = pb_st.tile([128, 1], f32, name="negm", tag="negm")
                            nc.vector.tensor_reduce(out=negm[:], in_=nm_pack[:],
                                                    axis=mybir.AxisListType.X,
                                                    op=mybir.AluOpType.min)
                            e_p = pb_st.tile([128, NQS], f32, name="e_p", tag="e_p")
                            nc.scalar.activation(e_p[:], nm_pack[:],
                                                 mybir.ActivationFunctionType.Exp,
                                                 scale=-1.0, bias=negm[:])
                            if pending_sm is not None:
                                softmax_tail(*pending_sm, on_act=(slot % 4 == 1))
                            pending_sm = (p_t, e_p, l_pack, pThb, tt)
                            av_step()
                            # deferred Phase-A group (qb0)
                            if deferred:
                                inject_deferred()
                            # qb0's proj+LN injected into qb1's early slots
                            if proj_state is not None and qb == 1 and 2 <= h <= 5:
                                pt = h - 2
                                if tt == 1:
                                    proj_prefetch(pt)
                                    proj_state[pt] = [proj_chunk(pt, 0, 512, pb_s, "ps_s")]
                                elif tt == 2:
                                    proj_state[pt].append(proj_chunk(pt, 512, 256, pb_s, "ps_s"))
                                    proj_finish(pt, proj_state.pop(pt))
                            slot += 1
                        av_queue.append([(g, r, h, qb, pThb, []), 0])
                    # qb end: flush last head's final tile tail + its AV
                    if pending_sm is not None:
                        softmax_tail(*pending_sm, on_act=False)
                        pending_sm = None
                    if qb == TQ // BLK - 1:
                        while av_queue:
                            av_step()
                        # pipelined tail: keep PE streaming pp chunks while the
                        # per-tile LN chains drain behind
                        t4 = qb * BLK
                        for tt in range(BLK):
                            proj_prefetch(t4 + tt)
                        pps = {}
                        pools = [(pb_x1, "ps_x1"), (pb_s, "ps_s")]
                        for tt in range(BLK):
                            pool, tg = pools[tt % 2]
                            pps[tt] = [proj_chunk(t4 + tt, 0, 512, pool, tg),
                                       proj_chunk(t4 + tt, 512, 256, pool, tg)]
                            if tt >= 1:
                                proj_finish(t4 + tt - 1, pps.pop(tt - 1), affine_dve=True)
                        proj_finish(t4 + BLK - 1, pps.pop(BLK - 1), affine_dve=True)
                        proj_state = None

    nc.compile()
    return nc


_CACHE = {}


def _get_program(cfg: Cfg):
    if cfg not in _CACHE:
        _CACHE[cfg] = build_program(cfg)
    return _CACHE[cfg]


def make_in_maps(x, qkv_w, proj_w, proj_b, ln_gamma, ln_beta, cfg: Cfg):
    """Host-side shard prep. Returns list of 8 in_maps."""
    C = cfg.C
    B = x.shape[0]
    wq_h = np.ascontiguousarray((qkv_w[0:C] * np.float32(cfg.D ** 0.5)).T.astype(np.float16))
    wk_h = np.ascontiguousarray(qkv_w[C:2 * C].T.astype(np.float16))
    wv_h = np.ascontiguousarray(qkv_w[2 * C:3 * C].T.astype(np.float16))
    wp_h = np.ascontiguousarray(proj_w.T.astype(np.float16))
    vecs = np.ascontiguousarray(np.stack([ln_gamma, ln_beta]).astype(np.float32))
    pb32 = proj_b.astype(np.float32)[None, :]
    in_maps = []
    for core in range(8):
        b, half = core // 2, core % 2
        b = min(b, B - 1)
        xb = np.asarray(x[b], dtype=np.float32)
        if half == 0:
            xkc = np.ascontiguousarray(xb)
        else:
            xkc = np.ascontiguousarray(np.concatenate([xb[cfg.Nq:], xb[:cfg.Nq]], axis=0))
        in_maps.append({"xh16": xkc.astype(np.float16),
                        "xq": np.ascontiguousarray(xkc[:cfg.Nq] + pb32),
                        "wq_h": wq_h, "wk_h": wk_h, "wv_h": wv_h,
                        "wp_h": wp_h, "vecs": vecs})
    return in_maps


def kernel(x, qkv_w, proj_w, proj_b, ln_gamma, ln_beta):
    from concourse.bass_utils import run_bass_kernel_spmd

    cfg = Cfg()
    nc = _get_program(cfg)
    x = np.asarray(x, dtype=np.float32)
    in_maps = make_in_maps(x, np.asarray(qkv_w, np.float32), np.asarray(proj_w, np.float32),
                           np.asarray(proj_b, np.float32), np.asarray(ln_gamma, np.float32),
                           np.asarray(ln_beta, np.float32), cfg)
    res = run_bass_kernel_spmd(nc, in_maps, core_ids=list(range(8)))
    B, N, C = x.shape
    out = np.empty((B, N, C), dtype=np.float32)
    for core in range(8):
        b, half = core // 2, core % 2
        out[b, half * cfg.Nq:(half + 1) * cfg.Nq] = res.results[core]["out"]
    return out


# revision 41
# speedup vs baseline: 1.0318x; 1.0318x over previous
"""Trainium2 Bass kernel for nn_ECA (attention block + residual + LayerNorm).

Reference computation (per batch b):
    qkv = x @ qkv_w.T ; q,k,v per head
    attn = softmax((q @ k.T) * sqrt(D))
    x1 = attn @ v  -> concat heads -> @ proj_w.T + proj_b
    out = LayerNorm(x + x1) * gamma + beta     # eps 1e-5

Sharding: 8 cores = 4 batches x 2 query-halves. Each core receives the full
batch's tokens ("xh16", rolled so its own 1024 query tokens are rows 0:1024),
computes K/V for all 2048 keys (duplicated across the 2 cores of a batch),
attention + proj + LN for its 1024 queries. No collectives.

v3: single-limb fp16 throughout (host numpy sim of the full fp16 pipeline:
rel err 3.7e-3 vs the 2e-2 gate). Q^T/K^T live in natural head-pair layout
qg/kg[g] [128 = dims of heads 2g,2g+1, N]; S is one K=64 matmul per 512-key
chunk (lhsT and rhs both at partition base r = (h*64)%128).

v4 scheduling: Phase A is split -- heads 0-5's K/Q/V are computed up front
(weights DMA'd on the ACT hwdge queue in parallel with the x^T transposes on
sync), heads 6-11's K/Q/V are deferred and injected one group per tile-slot
into qb0's attention loop, riding the pb_s psum rotation (third slot of the
q0/q1/def cycle). qb0's proj+LN is likewise injected into qb1's early head
slots (pp chunks ride the same rotation). The softmax normalize alternates
between DVE (tensor_scalar) and ACT (activation Identity, scale=e*rl) to
balance the two engines; AV psum->x1t copies go to ACT for even heads
(partition base 0) and DVE for odd heads (base 64 -- ACT cannot shift
partition base). Known HW landmines: tensor_tensor_reduce hangs the device,
ACT copies cannot shift partition base (DVE can), gpsimd tensor ops on tiny
tiles fault, SBUF<->SBUF DMAs deadlock against xbar transposes.
"""

import sys
from dataclasses import dataclass

import numpy as np

try:
    import concourse.bass  # noqa: F401
except ImportError:  # fresh dir without sitecustomize path
    sys.path.insert(0, "/opt/trn_rl_repo")


@dataclass(frozen=True)
class Cfg:
    Nk: int = 2048   # keys per core (full batch)
    Nq: int = 1024   # queries per core
    C: int = 768     # model dim (also total head dim H*D)
    H: int = 12
    D: int = 64
    lowp: str | None = None  # unused; kept for test harness compat

    @property
    def CH(self):
        return self.C // 128

    @property
    def G(self):
        return (self.H * self.D) // 128

    @property
    def TQ(self):
        return self.Nq // 128

    @property
    def TK(self):
        return self.Nk // 128

    @property
    def slabs(self):
        return self.Nk // self.Nq


def build_program(cfg: Cfg):
    from collections import deque

    import concourse.bass as bass
    import concourse.mybir as mybir
    import concourse.tile as tile

    from concourse import bacc

    f32 = mybir.dt.float32
    f16 = mybir.dt.float16
    ts = bass.ts
    Nk, Nq, C, H, D = cfg.Nk, cfg.Nq, cfg.C, cfg.H, cfg.D
    CH, G, TQ, TK = cfg.CH, cfg.G, cfg.TQ, cfg.TK
    QC = H * D
    assert QC % 128 == 0 and C % 128 == 0 and Nq % 128 == 0

    nc = bacc.Bacc("TRN2", target_bir_lowering=False, debug=False, num_devices=8)

    xh_d = nc.dram_tensor("xh16", [Nk, C], f16, kind="ExternalInput")
    xq_d = nc.dram_tensor("xq", [Nq, C], f32, kind="ExternalInput")  # x + proj_b
    wq_d = nc.dram_tensor("wq_h", [C, QC], f16, kind="ExternalInput")
    wk_d = nc.dram_tensor("wk_h", [C, QC], f16, kind="ExternalInput")
    wv_d = nc.dram_tensor("wv_h", [C, QC], f16, kind="ExternalInput")
    wp_d = nc.dram_tensor("wp_h", [QC, C], f16, kind="ExternalInput")
    vec_d = nc.dram_tensor("vecs", [2, C], f32, kind="ExternalInput")  # gamma, beta
    out_d = nc.dram_tensor("out", [Nq, C], f32, kind="ExternalOutput")

    J = 512              # matmul free-dim chunk (one psum bank)
    JQ = 1024            # softmax quarter width (2 psum banks)
    NQS = Nk // JQ       # quarters per row (2)
    BLK = min(4, TQ)     # q-tiles per AV block
    GLO = G // 2         # head-pair groups computed up front (g 0..2)

    with tile.TileContext(nc) as tc:
        with tc.tile_pool(name="persist", bufs=1) as persist:
            # kg[g]/qg[g]: K^T/Q^T in natural layout, rows = dims of heads
            # 2g (0:64) and 2g+1 (64:128).
            kg = [persist.tile([128, Nk], f16, name=f"kg{g}", tag=f"kg{g}")
                  for g in range(G)]
            qg = [persist.tile([128, Nq], f16, name=f"qg{g}", tag=f"qg{g}")
                  for g in range(G)]
            vb = [persist.tile([128, H, D], f16, name=f"vb{t}", tag=f"vb{t}") for t in range(TK)]
            # x^T persists into Phase B for the deferred K/Q/V groups
            xh2 = [persist.tile([128, CH, Nq], f16, name=f"xh{s}", tag=f"xh{s}")
                   for s in range(cfg.slabs)]

            def emit_kq_group(ps_pool, g, which, wgh, slab, j, on_dve=False):
                # one j-chunk of K^T (or Q^T): 6 matmuls + a cast-copy
                xh = xh2[slab]
                ps = ps_pool.tile([128, J], f32, name="ps_qk", tag=ps_pool._v4tag)
                for c in range(CH):
                    nc.tensor.matmul(ps[:], wgh[:, c, :], xh[:, c, ts(j, J)],
                                     start=(c == 0), stop=(c == CH - 1))
                cp = nc.vector.tensor_copy if on_dve else nc.scalar.copy
                if which == "k":
                    sl = slice(slab * Nq + j * J, slab * Nq + (j + 1) * J)
                    cp(kg[g][:, sl], ps[:])
                else:
                    cp(qg[g][:, ts(j, J)], ps[:])

            def emit_v_group(ps_pool, wvg, vc_base, slab, t, pw, on_dve=False):
                # V for one token tile: 6 matmuls + cast-copy (token-major)
                xh = xh2[slab]
                vw = 384
                psv = ps_pool.tile([128, pw], f32, name="psv", tag=ps_pool._v4tag)
                for c in range(CH):
                    nc.tensor.matmul(psv[:, :vw], xh[:, c, ts(t, 128)],
                                     wvg[:, c, :vw],
                                     start=(c == 0), stop=(c == CH - 1))
                hb = vc_base // D
                cp = nc.vector.tensor_copy if on_dve else nc.scalar.copy
                cp(vb[slab * TQ + t][:, hb:hb + vw // D, 0:D], psv[:, :vw])

            # ---------------- Phase A (inline half): x^T, g 0..2, V lo ----------------
            with tc.tile_pool(name="pa_w", bufs=6) as pa_w, \
                 tc.tile_pool(name="pa_wv", bufs=1) as pa_wv, \
                 tc.tile_pool(name="pa_ps", bufs=4, space="PSUM") as pa_ps, \
                 tc.tile_pool(name="pa_psv", bufs=4, space="PSUM") as pa_psv:
                pa_ps._v4tag = "ps_qk"
                pa_psv._v4tag = "psv"

                # weight DMAs go on sync BEFORE the x^T transposes (the ACT
                # queue is blocked by activation-table loads at startup)
                wlo = {}
                for g in range(GLO):
                    for which, w_d in (("k", wk_d), ("q", wq_d)):
                        wgh = pa_w.tile([128, CH, 128], f16, name="wgh", tag=f"w{which}{g}", bufs=1)
                        nc.sync.dma_start(wgh[:], w_d.ap()[:, ts(g, 128)].rearrange("(c p) n -> p c n", p=128))
                        wlo[(which, g)] = wgh
                wv_lo = pa_wv.tile([128, CH, 384], f16, name="wv_lo", tag="wv_lo")
                nc.scalar.dma_start(wv_lo[:], wv_d.ap()[:, 0:384].rearrange("(c p) n -> p c n", p=128))
                for slab in range(cfg.slabs):
                    for t in range(TQ):
                        row = slice((slab * TQ + t) * 128, (slab * TQ + t + 1) * 128)
                        nc.sync.dma_start(xh2[slab][:, :, ts(t, 128)], xh_d.ap()[row, :], transpose=True)

                # all slab-0 compute first (slab-1 transposes still landing)
                for g in range(GLO):
                    for j in range(Nq // J):
                        emit_kq_group(pa_ps, g, "k", wlo[("k", g)], 0, j)
                    for j in range(Nq // J):
                        emit_kq_group(pa_ps, g, "q", wlo[("q", g)], 0, j)
                for t in range(TQ):
                    emit_v_group(pa_psv, wv_lo, 0, 0, t, 384)
                for g in range(GLO):
                    for j in range(Nq // J):
                        emit_kq_group(pa_ps, g, "k", wlo[("k", g)], 1, j)
                for t in range(TQ):
                    emit_v_group(pa_psv, wv_lo, 0, 1, t, 384)

            # ---------------- Phase B: attention + proj + LN ----------------
            with tc.tile_pool(name="pc_w", bufs=1) as pc_w, \
                 tc.tile_pool(name="pd_w", bufs=4) as pd_w, \
                 tc.tile_pool(name="pb_p", bufs=5) as pb_p, \
                 tc.tile_pool(name="pb_pth", bufs=2) as pb_pth, \
                 tc.tile_pool(name="pb_st", bufs=6) as pb_st, \
                 tc.tile_pool(name="pc_sb", bufs=2) as pc_sb, \
                 tc.tile_pool(name="pc_st", bufs=3) as pc_st, \
                 tc.tile_pool(name="pb_s", bufs=3, space="PSUM") as pb_s, \
                 tc.tile_pool(name="pb_x1", bufs=2, space="PSUM") as pb_x1:
                pb_s._v4tag = "ps_s"
                x1t = [pc_w.tile([128, Nq], f16, name=f"x1t{g}", tag=f"x1t{g}")
                       for g in range(G)]

                # LN/proj prep
                ones = pc_w.tile([1, 128], f32, name="ones", tag="ones")
                nc.gpsimd.memset(ones[:], 1.0)
                bc = []
                for vi in range(2):
                    vrow = pc_w.tile([1, C], f32, name=f"vrow{vi}", tag=f"vrow{vi}")
                    nc.sync.dma_start(vrow[:], vec_d.ap()[vi:vi + 1, :])
                    bct = pc_w.tile([128, C], f32, name=f"bc{vi}", tag=f"bc{vi}")
                    for j in range(0, C, 512):
                        w = min(512, C - j)
                        psb = pb_x1.tile([128, 512], f32, name="psb", tag="ps_x1")
                        nc.tensor.matmul(psb[:, :w], ones[:], vrow[:, j:j + w],
                                         start=True, stop=True)
                        nc.scalar.copy(bct[:, j:j + w], psb[:, :w])
                    bc.append(bct)
                gam_bc, bet_bc = bc
                wpb = []
                for c in range(G):
                    wpc = pc_w.tile([128, C], f16, name=f"wpb{c}", tag=f"wpb{c}")
                    nc.sync.dma_start(wpc[:], wp_d.ap()[ts(c, 128), :])
                    wpb.append(wpc)
                eps_t = pc_w.tile([128, 1], f32, name="eps_t", tag="eps_t")
                nc.gpsimd.memset(eps_t[:], 1e-5)

                # -------- deferred Phase A (g 3..5 K/Q + V hi), injected --------
                wdef = {}

                def d_dma_w(which, g):
                    def run():
                        w_d = wk_d if which == "k" else wq_d
                        wgh = pd_w.tile([128, CH, 128], f16, name="wgh_d", tag="wd", bufs=2)
                        nc.gpsimd.dma_start(wgh[:], w_d.ap()[:, ts(g, 128)].rearrange("(c p) n -> p c n", p=128))
                        wdef[(which, g)] = wgh
                    return run, False

                def d_dma_wv():
                    def run():
                        wvg = pd_w.tile([128, CH, 384], f16, name="wv_hi", tag="wv_hi", bufs=1)
                        nc.gpsimd.dma_start(wvg[:], wv_d.ap()[:, 384:768].rearrange("(c p) n -> p c n", p=128))
                        wdef["v"] = wvg
                    return run, False

                dctr = [0]

                def d_kq(which, g, slab):
                    def run():
                        for j in range(Nq // J):
                            dctr[0] += 1
                            emit_kq_group(pb_s, g, which, wdef[(which, g)], slab, j,
                                          on_dve=(dctr[0] % 2 == 0))
                    return run, True

                def d_v(slab, t):
                    def run():
                        dctr[0] += 1
                        emit_v_group(pb_s, wdef["v"], 384, slab, t, J,
                                     on_dve=(dctr[0] % 2 == 0))
                    return run, True

                deferred = deque()
                for g in range(GLO, G):
                    deferred.append(d_dma_w("k", g))
                    deferred.append(d_kq("k", g, 0))
                    deferred.append(d_kq("k", g, 1))
                    deferred.append(d_dma_w("q", g))
                    deferred.append(d_kq("q", g, 0))
                    if g == GLO:
                        deferred.append(d_dma_wv())
                    for t in range(2):
                        deferred.append(d_v(0, (g - GLO) * 2 + t))
                for t in range(2 * (G - GLO), TQ):
                    deferred.append(d_v(0, t))
                for t in range(TQ):
                    deferred.append(d_v(1, t))

                def inject_deferred():
                    # run dma thunks for free; one compute group per call
                    while deferred:
                        run, is_compute = deferred.popleft()
                        run()
                        if is_compute:
                            break

                KCH = TK // BLK  # AV k-chunks per tile slot

                def emit_av_chunk(st, kc):
                    # one quarter of the AV accumulation; spread one chunk per
                    # tile slot, starting a slot AFTER the head boundary so the
                    # last tile's normalize+transposes have landed by then
                    g, r, h, qb, pThb, holder = st
                    if kc == 0:
                        holder.append(pb_x1.tile([D, BLK * 128], f32, name="ps_x1", tag="ps_x1"))
                    ps_x1 = holder[0]
                    for k in range(kc * KCH, (kc + 1) * KCH):
                        nc.tensor.matmul(ps_x1[:],
                                         vb[k][:, h, 0:D],
                                         pThb[:, k, :, :].rearrange("p t q -> p (t q)"),
                                         start=(k == 0), stop=(k == TK - 1))
                    if kc == BLK - 1:
                        dst = x1t[g][r:r + D, qb * BLK * 128:(qb + 1) * BLK * 128]
                        if r == 0:
                            nc.scalar.copy(dst, ps_x1[:])  # no partition shift
                        else:
                            nc.vector.tensor_copy(dst, ps_x1[:])

                av_queue = deque()  # entries: [st, next_kc]

                def av_step():
                    if av_queue:
                        ent = av_queue[0]
                        emit_av_chunk(ent[0], ent[1])
                        ent[1] += 1
                        if ent[1] == BLK:
                            av_queue.popleft()

                KQ = JQ // 128

                def softmax_tail(p_t, e_p, l_pack, pThb, tt, on_act):
                    # l = sum_j l_j e_j (one fused stt with accumulate);
                    # p_n = p_j * e_j / l. Issued one tile LATE so these small
                    # ops never block the next tile's big maxes/exps at the
                    # FIFO heads.
                    lw = pb_st.tile([128, NQS], f32, name="lw", tag="lw")
                    l_tot = pb_st.tile([128, 1], f32, name="l_tot", tag="l_tot")
                    nc.vector.scalar_tensor_tensor(
                        out=lw[:], in0=l_pack[:], scalar=1.0, in1=e_p[:],
                        op0=mybir.AluOpType.mult, op1=mybir.AluOpType.mult,
                        accum_out=l_tot[:])
                    rl = pb_st.tile([128, 1], f32, name="rl", tag="rl")
                    nc.vector.reciprocal(rl[:], l_tot[:])
                    p_n = pb_p.tile([128, Nk], f16, name="p_n", tag="p_t")
                    if on_act:
                        er = pb_st.tile([128, NQS], f32, name="er", tag="er")
                        nc.vector.tensor_scalar(out=er[:], in0=e_p[:], scalar1=rl[:],
                                                scalar2=None,
                                                op0=mybir.AluOpType.mult)
                    for j2 in range(NQS):
                        if on_act:
                            nc.scalar.activation(p_n[:, ts(j2, JQ)], p_t[:, ts(j2, JQ)],
                                                 mybir.ActivationFunctionType.Identity,
                                                 scale=er[:, j2:j2 + 1])
                        else:
                            nc.vector.tensor_scalar(
                                out=p_n[:, ts(j2, JQ)], in0=p_t[:, ts(j2, JQ)],
                                scalar1=e_p[:, j2:j2 + 1], scalar2=rl[:],
                                op0=mybir.AluOpType.mult, op1=mybir.AluOpType.mult)
                        # per-quarter transpose: lower latency, same bytes
                        nc.sync.dma_start(pThb[:, j2 * KQ:(j2 + 1) * KQ, tt, :],
                                          p_n[:, ts(j2, JQ)], transpose=True)

                # ---- proj + residual + LayerNorm, per tile, injectable ----
                NSTAT = 256
                nsub = C // NSTAT
                xr_t = {}

                def proj_prefetch(t):
                    xr = pc_sb.tile([128, C], f16, name="xr", tag="xrh", bufs=4)
                    nc.gpsimd.dma_start(xr[:], xq_d.ap()[ts(t, 128), :])
                    xr_t[t] = xr

                def proj_chunk(t, j, w, pool, tag):
                    pp = pool.tile([128, w], f32, name="pp", tag=tag)
                    for c in range(G):
                        nc.tensor.matmul(pp[:, :w], x1t[c][:, ts(t, 128)], wpb[c][:, j:j + w],
                                         start=(c == 0), stop=(c == G - 1))
                    return (j, w, pp)

                def proj_finish(t, pps, affine_dve=False):
                    xr = xr_t.pop(t)
                    u = pc_sb.tile([128, C], f32, name="u", tag="u")
                    for (j, w, pp) in pps:
                        nc.vector.tensor_add(u[:, j:j + w], pp[:, :w], xr[:, j:j + w])
                    stats = pc_st.tile([128, nsub, 6], f32, name="stats", tag="stats")
                    for s in range(nsub):
                        nc.vector.bn_stats(out=stats[:, s, :], in_=u[:, ts(s, NSTAT)])
                    mv = pc_st.tile([128, 2], f32, name="mv", tag="mv")
                    nc.vector.bn_aggr(out=mv[:], in_=stats[:])
                    rstd = pc_st.tile([128, 1], f32, name="rstd", tag="rstd")
                    nc.scalar.activation(rstd[:], mv[:, 1:2],
                                         mybir.ActivationFunctionType.Sqrt, bias=eps_t[:])
                    nc.vector.reciprocal(rstd[:], rstd[:])
                    nmr = pc_st.tile([128, 1], f32, name="nmr", tag="nmr")
                    nc.vector.tensor_scalar(out=nmr[:], in0=mv[:, 0:1],
                                            scalar1=rstd[:], scalar2=-1.0,
                                            op0=mybir.AluOpType.mult,
                                            op1=mybir.AluOpType.mult)
                    of = pc_sb.tile([128, C], f32, name="of", tag="u")
                    # (u - mu)*rstd on ACT, then *gamma, +beta on GpSimd
                    nc.scalar.activation(of[:], u[:],
                                         mybir.ActivationFunctionType.Identity,
                                         scale=rstd[:], bias=nmr[:])
                    if affine_dve:
                        # final-flush tiles: gpsimd chains would serialize the
                        # tail; DVE is idle there
                        nc.vector.tensor_mul(of[:], of[:], gam_bc[:])
                        nc.vector.tensor_add(of[:], of[:], bet_bc[:])
                    else:
                        nc.gpsimd.tensor_mul(of[:], of[:], gam_bc[:])
                        nc.gpsimd.tensor_add(of[:], of[:], bet_bc[:])
                    nc.sync.dma_start(out_d.ap()[ts(t, 128), :], of[:])

                def proj_full(t):
                    proj_prefetch(t)
                    pps = [proj_chunk(t, 0, 512, pb_x1, "ps_x1"),
                           proj_chunk(t, 512, 256, pb_x1, "ps_x1")]
                    proj_finish(t, pps)

                pending_sm = None
                slot = 0
                proj_state = {}  # t -> list of pps
                for qb in range(TQ // BLK):
                    for h in range(H):
                        g, r = divmod(h * D, 128)
                        for tt in range(BLK):
                            t = qb * BLK + tt
                            q_s = qg[g][r:r + D, ts(t, 128)]
                            p_t = pb_p.tile([128, Nk], f16, name="p_t", tag="p_t")
                            nm_pack = pb_st.tile([128, NQS], f32, name="nm_pack", tag="nm_pack")
                            l_pack = pb_st.tile([128, NQS], f32, name="l_pack", tag="l_pack")
                            for j2 in range(NQS):
                                ps_s = pb_s.tile([128, JQ], f32, name="ps_s", tag="ps_s")
                                for jj in range(JQ // J):
                                    sl = slice(j2 * JQ + jj * J, j2 * JQ + (jj + 1) * J)
                                    nc.tensor.matmul(ps_s[:, ts(jj, J)], q_s,
                                                     kg[g][r:r + D, sl],
                                                     start=True, stop=True)
                                nc.vector.reduce_max(out=nm_pack[:, j2:j2 + 1], in_=ps_s[:],
                                                     axis=mybir.AxisListType.X, negate=True)
                                nc.scalar.activation(p_t[:, ts(j2, JQ)], ps_s[:],
                                                     mybir.ActivationFunctionType.Exp,
                                                     bias=nm_pack[:, j2:j2 + 1],
                                                     accum_out=l_pack[:, j2:j2 + 1])
                            if tt == 0:
                                pThb = pb_pth.tile([128, TK, BLK, 128], f16, name="pThb", tag="pThb")
                            # negm/e_p stay in-tile (depend only on this tile's
                            # maxes); the lagged tail runs next tile
                            negm = pb_st.tile([128, 1], f32, name="negm", tag="negm")
                            nc.vector.tensor_reduce(out=negm[:], in_=nm_pack[:],
                                                    axis=mybir.AxisListType.X,
                                                    op=mybir.AluOpType.min)
                            e_p = pb_st.tile([128, NQS], f32, name="e_p", tag="e_p")
                            nc.scalar.activation(e_p[:], nm_pack[:],
                                                 mybir.ActivationFunctionType.Exp,
                                                 scale=-1.0, bias=negm[:])
                            if pending_sm is not None:
                                softmax_tail(*pending_sm, on_act=(slot % 4 == 1))
                            pending_sm = (p_t, e_p, l_pack, pThb, tt)
                            av_step()
                            # deferred Phase-A group (qb0)
                            if deferred:
                                inject_deferred()
                            # qb0's proj+LN injected into qb1's early slots
                            if proj_state is not None and qb == 1 and 2 <= h <= 5:
                                pt = h - 2
                                if tt == 1:
                                    proj_prefetch(pt)
                                    proj_state[pt] = [proj_chunk(pt, 0, 512, pb_s, "ps_s")]
                                elif tt == 2:
                                    proj_state[pt].append(proj_chunk(pt, 512, 256, pb_s, "ps_s"))
                                    proj_finish(pt, proj_state.pop(pt))
                            slot += 1
                        av_queue.append([(g, r, h, qb, pThb, []), 0])
                    # qb end: flush last head's final tile tail + its AV
                    if pending_sm is not None:
                        softmax_tail(*pending_sm, on_act=False)
                        pending_sm = None
                    if qb == TQ // BLK - 1:
                        while av_queue:
                            av_step()
                        # pipelined tail: keep PE streaming pp chunks while the
                        # per-tile LN chains drain behind
                        t4 = qb * BLK
                        for tt in range(BLK):
                            proj_prefetch(t4 + tt)
                        pps = {}
                        pools = [(pb_x1, "ps_x1"), (pb_s, "ps_s")]
                        for tt in range(BLK):
                            pool, tg = pools[tt % 2]
                            pps[tt] = [proj_chunk(t4 + tt, 0, 512, pool, tg),
                                       proj_chunk(t4 + tt, 512, 256, pool, tg)]
                            if tt >= 1:
                                proj_finish(t4 + tt - 1, pps.pop(tt - 1), affine_dve=True)
                        proj_finish(t4 + BLK - 1, pps.pop(BLK - 1), affine_dve=True)
                        proj_state = None

    nc.compile()
    return nc


_CACHE = {}


def _get_program(cfg: Cfg):
    if cfg not in _CACHE:
        _CACHE[cfg] = build_program(cfg)
    return _CACHE[cfg]


def make_in_maps(x, qkv_w, proj_w, proj_b, ln_gamma, ln_beta, cfg: Cfg):
    """Host-side shard prep. Returns list of 8 in_maps."""
    C = cfg.C
    B = x.shape[0]
    wq_h = np.ascontiguousarray((qkv_w[0:C] * np.float32(cfg.D ** 0.5)).T.astype(np.float16))
    wk_h = np.ascontiguousarray(qkv_w[C:2 * C].T.astype(np.float16))
    wv_h = np.ascontiguousarray(qkv_w[2 * C:3 * C].T.astype(np.float16))
    wp_h = np.ascontiguousarray(proj_w.T.astype(np.float16))
    vecs = np.ascontiguousarray(np.stack([ln_gamma, ln_beta]).astype(np.float32))
    pb32 = proj_b.astype(np.float32)[None, :]
    in_maps = []
    for core in range(8):
        b, half = core // 2, core % 2
        b = min(b, B - 1)
        xb = np.asarray(x[b], dtype=np.float32)
        if half == 0:
            xkc = np.ascontiguousarray(xb)
        else:
            xkc = np.ascontiguousarray(np.concatenate([xb[cfg.Nq:], xb[:cfg.Nq]], axis=0))
        in_maps.append({"xh16": xkc.astype(np.float16),
                        "xq": np.ascontiguousarray(xkc[:cfg.Nq] + pb32),
                        "wq_h": wq_h, "wk_h": wk_h, "wv_h": wv_h,
                        "wp_h": wp_h, "vecs": vecs})
    return in_maps


def kernel(x, qkv_w, proj_w, proj_b, ln_gamma, ln_beta):
    from concourse.bass_utils import run_bass_kernel_spmd

    cfg = Cfg()
    nc = _get_program(cfg)
    x = np.asarray(x, dtype=np.float32)
    in_maps = make_in_maps(x, np.asarray(qkv_w, np.float32), np.asarray(proj_w, np.float32),
                           np.asarray(proj_b, np.float32), np.asarray(ln_gamma, np.float32),
                           np.asarray(ln_beta, np.float32), cfg)
    res = run_bass_kernel_spmd(nc, in_maps, core_ids=list(range(8)))
    B, N, C = x.shape
    out = np.empty((B, N, C), dtype=np.float32)
    for core in range(8):
        b, half = core // 2, core % 2
        out[b, half * cfg.Nq:(half + 1) * cfg.Nq] = res.results[core]["out"]
    return out


# revision 42
# speedup vs baseline: 1.0343x; 1.0025x over previous
"""Trainium2 Bass kernel for nn_ECA (attention block + residual + LayerNorm).

Reference computation (per batch b):
    qkv = x @ qkv_w.T ; q,k,v per head
    attn = softmax((q @ k.T) * sqrt(D))
    x1 = attn @ v  -> concat heads -> @ proj_w.T + proj_b
    out = LayerNorm(x + x1) * gamma + beta     # eps 1e-5

Sharding: 8 cores = 4 batches x 2 query-halves. Each core receives the full
batch's tokens ("xh16", rolled so its own 1024 query tokens are rows 0:1024),
computes K/V for all 2048 keys (duplicated across the 2 cores of a batch),
attention + proj + LN for its 1024 queries. No collectives.

v3: single-limb fp16 throughout (host numpy sim of the full fp16 pipeline:
rel err 3.7e-3 vs the 2e-2 gate). Q^T/K^T live in natural head-pair layout
qg/kg[g] [128 = dims of heads 2g,2g+1, N]; S is one K=64 matmul per 512-key
chunk (lhsT and rhs both at partition base r = (h*64)%128).

Scheduling: Phase A is split -- heads 0-5's K/Q/V are computed up front
(weight DMAs issued on sync ahead of the x^T transposes; the ACT queue is
blocked by activation-table loads at startup), heads 6-11's K/Q/V are
deferred and injected one group per tile-slot into qb0's attention loop,
riding the pb_s psum rotation (third slot of the q0/q1/def cycle). qb0's
proj+LN is injected into qb1's head-2..5 slots (pp chunks ride the same
rotation); the final qb's proj+LN tail is pipelined across both psum pools
with its LN affine on DVE (gpsimd chains would serialize it). AV runs as a
one-chunk-per-slot queue, starting one slot after the head boundary so the
previous head's normalize+transposes have landed. The softmax tail computes
l = sum_j l_j e_j in a single scalar_tensor_tensor with accum_out, and the
normalize alternates DVE (tensor_scalar, 3/4 of slots) and ACT (activation
Identity scale=e*rl) to balance the engines; AV psum->x1t copies go to ACT
for even heads (partition base 0) and DVE for odd heads (base 64 -- ACT
cannot shift partition base). Known HW landmines: tensor_tensor_reduce hangs
the device, ACT copies cannot shift partition base (DVE can), gpsimd tensor
ops on tiny tiles fault, gpsimd tensor_scalar is ~30x slower than DVE,
SBUF<->SBUF DMAs deadlock against xbar transposes, AluOpType.divide fails at
runtime on DVE.
"""

import sys
from dataclasses import dataclass

import numpy as np

try:
    import concourse.bass  # noqa: F401
except ImportError:  # fresh dir without sitecustomize path
    sys.path.insert(0, "/opt/trn_rl_repo")


@dataclass(frozen=True)
class Cfg:
    Nk: int = 2048   # keys per core (full batch)
    Nq: int = 1024   # queries per core
    C: int = 768     # model dim (also total head dim H*D)
    H: int = 12
    D: int = 64
    lowp: str | None = None  # unused; kept for test harness compat

    @property
    def CH(self):
        return self.C // 128

    @property
    def G(self):
        return (self.H * self.D) // 128

    @property
    def TQ(self):
        return self.Nq // 128

    @property
    def TK(self):
        return self.Nk // 128

    @property
    def slabs(self):
        return self.Nk // self.Nq


def build_program(cfg: Cfg):
    from collections import deque

    import concourse.bass as bass
    import concourse.mybir as mybir
    import concourse.tile as tile

    from concourse import bacc

    f32 = mybir.dt.float32
    f16 = mybir.dt.float16
    ts = bass.ts
    Nk, Nq, C, H, D = cfg.Nk, cfg.Nq, cfg.C, cfg.H, cfg.D
    CH, G, TQ, TK = cfg.CH, cfg.G, cfg.TQ, cfg.TK
    QC = H * D
    assert QC % 128 == 0 and C % 128 == 0 and Nq % 128 == 0

    nc = bacc.Bacc("TRN2", target_bir_lowering=False, debug=False, num_devices=8)

    xh_d = nc.dram_tensor("xh16", [Nk, C], f16, kind="ExternalInput")
    xq_d = nc.dram_tensor("xq", [Nq, C], f32, kind="ExternalInput")  # x + proj_b
    wq_d = nc.dram_tensor("wq_h", [C, QC], f16, kind="ExternalInput")
    wk_d = nc.dram_tensor("wk_h", [C, QC], f16, kind="ExternalInput")
    wv_d = nc.dram_tensor("wv_h", [C, QC], f16, kind="ExternalInput")
    wp_d = nc.dram_tensor("wp_h", [QC, C], f16, kind="ExternalInput")
    vec_d = nc.dram_tensor("vecs", [2, C], f32, kind="ExternalInput")  # gamma, beta
    out_d = nc.dram_tensor("out", [Nq, C], f32, kind="ExternalOutput")

    J = 512              # matmul free-dim chunk (one psum bank)
    JQ = 1024            # softmax quarter width (2 psum banks)
    NQS = Nk // JQ       # quarters per row (2)
    BLK = min(4, TQ)     # q-tiles per AV block
    GLO = G // 2         # head-pair groups computed up front (g 0..2)

    with tile.TileContext(nc) as tc:
        with tc.tile_pool(name="persist", bufs=1) as persist:
            # kg[g]/qg[g]: K^T/Q^T in natural layout, rows = dims of heads
            # 2g (0:64) and 2g+1 (64:128).
            kg = [persist.tile([128, Nk], f16, name=f"kg{g}", tag=f"kg{g}")
                  for g in range(G)]
            qg = [persist.tile([128, Nq], f16, name=f"qg{g}", tag=f"qg{g}")
                  for g in range(G)]
            vb = [persist.tile([128, H, D], f16, name=f"vb{t}", tag=f"vb{t}") for t in range(TK)]
            # x^T persists into Phase B for the deferred K/Q/V groups
            xh2 = [persist.tile([128, CH, Nq], f16, name=f"xh{s}", tag=f"xh{s}")
                   for s in range(cfg.slabs)]

            def emit_kq_group(ps_pool, g, which, wgh, slab, j, on_dve=False):
                # one j-chunk of K^T (or Q^T): 6 matmuls + a cast-copy
                xh = xh2[slab]
                ps = ps_pool.tile([128, J], f32, name="ps_qk", tag=ps_pool._v4tag)
                for c in range(CH):
                    nc.tensor.matmul(ps[:], wgh[:, c, :], xh[:, c, ts(j, J)],
                                     start=(c == 0), stop=(c == CH - 1))
                cp = nc.vector.tensor_copy if on_dve else nc.scalar.copy
                if which == "k":
                    sl = slice(slab * Nq + j * J, slab * Nq + (j + 1) * J)
                    cp(kg[g][:, sl], ps[:])
                else:
                    cp(qg[g][:, ts(j, J)], ps[:])

            def emit_v_group(ps_pool, wvg, vc_base, slab, t, pw, on_dve=False):
                # V for one token tile: 6 matmuls + cast-copy (token-major)
                xh = xh2[slab]
                vw = 384
                psv = ps_pool.tile([128, pw], f32, name="psv", tag=ps_pool._v4tag)
                for c in range(CH):
                    nc.tensor.matmul(psv[:, :vw], xh[:, c, ts(t, 128)],
                                     wvg[:, c, :vw],
                                     start=(c == 0), stop=(c == CH - 1))
                hb = vc_base // D
                cp = nc.vector.tensor_copy if on_dve else nc.scalar.copy
                cp(vb[slab * TQ + t][:, hb:hb + vw // D, 0:D], psv[:, :vw])

            # ---------------- Phase A (inline half): x^T, g 0..2, V lo ----------------
            with tc.tile_pool(name="pa_w", bufs=6) as pa_w, \
                 tc.tile_pool(name="pa_wv", bufs=1) as pa_wv, \
                 tc.tile_pool(name="pa_ps", bufs=4, space="PSUM") as pa_ps, \
                 tc.tile_pool(name="pa_psv", bufs=4, space="PSUM") as pa_psv:
                pa_ps._v4tag = "ps_qk"
                pa_psv._v4tag = "psv"

                # weight DMAs go on sync BEFORE the x^T transposes (the ACT
                # queue is blocked by activation-table loads at startup)
                wlo = {}
                for g in range(GLO):
                    for which, w_d in (("k", wk_d), ("q", wq_d)):
                        wgh = pa_w.tile([128, CH, 128], f16, name="wgh", tag=f"w{which}{g}", bufs=1)
                        nc.sync.dma_start(wgh[:], w_d.ap()[:, ts(g, 128)].rearrange("(c p) n -> p c n", p=128))
                        wlo[(which, g)] = wgh
                wv_lo = pa_wv.tile([128, CH, 384], f16, name="wv_lo", tag="wv_lo")
                nc.scalar.dma_start(wv_lo[:], wv_d.ap()[:, 0:384].rearrange("(c p) n -> p c n", p=128))
                for slab in range(cfg.slabs):
                    for t in range(TQ):
                        row = slice((slab * TQ + t) * 128, (slab * TQ + t + 1) * 128)
                        nc.sync.dma_start(xh2[slab][:, :, ts(t, 128)], xh_d.ap()[row, :], transpose=True)

                # all slab-0 compute first (slab-1 transposes still landing)
                for g in range(GLO):
                    for j in range(Nq // J):
                        emit_kq_group(pa_ps, g, "k", wlo[("k", g)], 0, j)
                    for j in range(Nq // J):
                        emit_kq_group(pa_ps, g, "q", wlo[("q", g)], 0, j)
                for t in range(TQ):
                    emit_v_group(pa_psv, wv_lo, 0, 0, t, 384)
                for g in range(GLO):
                    for j in range(Nq // J):
                        emit_kq_group(pa_ps, g, "k", wlo[("k", g)], 1, j)
                for t in range(TQ):
                    emit_v_group(pa_psv, wv_lo, 0, 1, t, 384)

            # ---------------- Phase B: attention + proj + LN ----------------
            with tc.tile_pool(name="pc_w", bufs=1) as pc_w, \
                 tc.tile_pool(name="pd_w", bufs=4) as pd_w, \
                 tc.tile_pool(name="pb_p", bufs=5) as pb_p, \
                 tc.tile_pool(name="pb_pth", bufs=2) as pb_pth, \
                 tc.tile_pool(name="pb_st", bufs=6) as pb_st, \
                 tc.tile_pool(name="pc_sb", bufs=2) as pc_sb, \
                 tc.tile_pool(name="pc_st", bufs=3) as pc_st, \
                 tc.tile_pool(name="pb_s", bufs=3, space="PSUM") as pb_s, \
                 tc.tile_pool(name="pb_x1", bufs=2, space="PSUM") as pb_x1:
                pb_s._v4tag = "ps_s"
                x1t = [pc_w.tile([128, Nq], f16, name=f"x1t{g}", tag=f"x1t{g}")
                       for g in range(G)]

                # LN/proj prep
                ones = pc_w.tile([1, 128], f32, name="ones", tag="ones")
                nc.gpsimd.memset(ones[:], 1.0)
                bc = []
                for vi in range(2):
                    vrow = pc_w.tile([1, C], f32, name=f"vrow{vi}", tag=f"vrow{vi}")
                    nc.sync.dma_start(vrow[:], vec_d.ap()[vi:vi + 1, :])
                    bct = pc_w.tile([128, C], f32, name=f"bc{vi}", tag=f"bc{vi}")
                    for j in range(0, C, 512):
                        w = min(512, C - j)
                        psb = pb_x1.tile([128, 512], f32, name="psb", tag="ps_x1")
                        nc.tensor.matmul(psb[:, :w], ones[:], vrow[:, j:j + w],
                                         start=True, stop=True)
                        nc.scalar.copy(bct[:, j:j + w], psb[:, :w])
                    bc.append(bct)
                gam_bc, bet_bc = bc
                wpb = []
                for c in range(G):
                    wpc = pc_w.tile([128, C], f16, name=f"wpb{c}", tag=f"wpb{c}")
                    nc.sync.dma_start(wpc[:], wp_d.ap()[ts(c, 128), :])
                    wpb.append(wpc)
                eps_t = pc_w.tile([128, 1], f32, name="eps_t", tag="eps_t")
                nc.gpsimd.memset(eps_t[:], 1e-5)

                # -------- deferred Phase A (g 3..5 K/Q + V hi), injected --------
                wdef = {}

                def d_dma_w(which, g):
                    def run():
                        w_d = wk_d if which == "k" else wq_d
                        wgh = pd_w.tile([128, CH, 128], f16, name="wgh_d", tag="wd", bufs=2)
                        nc.gpsimd.dma_start(wgh[:], w_d.ap()[:, ts(g, 128)].rearrange("(c p) n -> p c n", p=128))
                        wdef[(which, g)] = wgh
                    return run, False

                def d_dma_wv():
                    def run():
                        wvg = pd_w.tile([128, CH, 384], f16, name="wv_hi", tag="wv_hi", bufs=1)
                        nc.gpsimd.dma_start(wvg[:], wv_d.ap()[:, 384:768].rearrange("(c p) n -> p c n", p=128))
                        wdef["v"] = wvg
                    return run, False

                dctr = [0]

                def d_kq(which, g, slab):
                    def run():
                        for j in range(Nq // J):
                            dctr[0] += 1
                            emit_kq_group(pb_s, g, which, wdef[(which, g)], slab, j,
                                          on_dve=(dctr[0] % 2 == 0))
                    return run, True

                def d_v(slab, t):
                    def run():
                        dctr[0] += 1
                        emit_v_group(pb_s, wdef["v"], 384, slab, t, J,
                                     on_dve=(dctr[0] % 2 == 0))
                    return run, True

                deferred = deque()
                for g in range(GLO, G):
                    deferred.append(d_dma_w("k", g))
                    deferred.append(d_kq("k", g, 0))
                    deferred.append(d_kq("k", g, 1))
                    deferred.append(d_dma_w("q", g))
                    deferred.append(d_kq("q", g, 0))
                    if g == GLO:
                        deferred.append(d_dma_wv())
                    for t in range(2):
                        deferred.append(d_v(0, (g - GLO) * 2 + t))
                for t in range(2 * (G - GLO), TQ):
                    deferred.append(d_v(0, t))
                for t in range(TQ):
                    deferred.append(d_v(1, t))

                def inject_deferred():
                    # run dma thunks for free; one compute group per call
                    while deferred:
                        run, is_compute = deferred.popleft()
                        run()
                        if is_compute:
                            break

                KCH = TK // BLK  # AV k-chunks per tile slot

                def emit_av_chunk(st, kc):
                    # one quarter of the AV accumulation; spread one chunk per
                    # tile slot, starting a slot AFTER the head boundary so the
                    # last tile's normalize+transposes have landed by then
                    g, r, h, qb, pThb, holder = st
                    if kc == 0:
                        holder.append(pb_x1.tile([D, BLK * 128], f32, name="ps_x1", tag="ps_x1"))
                    ps_x1 = holder[0]
                    for k in range(kc * KCH, (kc + 1) * KCH):
                        nc.tensor.matmul(ps_x1[:],
                                         vb[k][:, h, 0:D],
                                         pThb[:, k, :, :].rearrange("p t q -> p (t q)"),
                                         start=(k == 0), stop=(k == TK - 1))
                    if kc == BLK - 1:
                        dst = x1t[g][r:r + D, qb * BLK * 128:(qb + 1) * BLK * 128]
                        if r == 0:
                            nc.scalar.copy(dst, ps_x1[:])  # no partition shift
                        else:
                            nc.vector.tensor_copy(dst, ps_x1[:])

                av_queue = deque()  # entries: [st, next_kc]

                def av_step():
                    if av_queue:
                        ent = av_queue[0]
                        emit_av_chunk(ent[0], ent[1])
                        ent[1] += 1
                        if ent[1] == BLK:
                            av_queue.popleft()

                KQ = JQ // 128

                def softmax_tail(p_t, e_p, l_pack, pThb, tt, on_act):
                    # l = sum_j l_j e_j (one fused stt with accumulate);
                    # p_n = p_j * e_j / l. Issued one tile LATE so these small
                    # ops never block the next tile's big maxes/exps at the
                    # FIFO heads.
                    lw = pb_st.tile([128, NQS], f32, name="lw", tag="lw")
                    l_tot = pb_st.tile([128, 1], f32, name="l_tot", tag="l_tot")
                    nc.vector.scalar_tensor_tensor(
                        out=lw[:], in0=l_pack[:], scalar=1.0, in1=e_p[:],
                        op0=mybir.AluOpType.mult, op1=mybir.AluOpType.mult,
                        accum_out=l_tot[:])
                    rl = pb_st.tile([128, 1], f32, name="rl", tag="rl")
                    nc.vector.reciprocal(rl[:], l_tot[:])
                    p_n = pb_p.tile([128, Nk], f16, name="p_n", tag="p_t")
                    if on_act:
                        er = pb_st.tile([128, NQS], f32, name="er", tag="er")
                        nc.vector.tensor_scalar(out=er[:], in0=e_p[:], scalar1=rl[:],
                                                scalar2=None,
                                                op0=mybir.AluOpType.mult)
                    for j2 in range(NQS):
                        if on_act:
                            nc.scalar.activation(p_n[:, ts(j2, JQ)], p_t[:, ts(j2, JQ)],
                                                 mybir.ActivationFunctionType.Identity,
                                                 scale=er[:, j2:j2 + 1])
                        else:
                            nc.vector.tensor_scalar(
                                out=p_n[:, ts(j2, JQ)], in0=p_t[:, ts(j2, JQ)],
                                scalar1=e_p[:, j2:j2 + 1], scalar2=rl[:],
                                op0=mybir.AluOpType.mult, op1=mybir.AluOpType.mult)
                        # per-quarter transpose: lower latency, same bytes
                        nc.sync.dma_start(pThb[:, j2 * KQ:(j2 + 1) * KQ, tt, :],
                                          p_n[:, ts(j2, JQ)], transpose=True)

                # ---- proj + residual + LayerNorm, per tile, injectable ----
                NSTAT = 256
                nsub = C // NSTAT
                xr_t = {}

                def proj_prefetch(t):
                    xr = pc_sb.tile([128, C], f16, name="xr", tag="xrh", bufs=4)
                    nc.gpsimd.dma_start(xr[:], xq_d.ap()[ts(t, 128), :])
                    xr_t[t] = xr

                def proj_chunk(t, j, w, pool, tag):
                    pp = pool.tile([128, w], f32, name="pp", tag=tag)
                    for c in range(G):
                        nc.tensor.matmul(pp[:, :w], x1t[c][:, ts(t, 128)], wpb[c][:, j:j + w],
                                         start=(c == 0), stop=(c == G - 1))
                    return (j, w, pp)

                def proj_finish(t, pps, affine_dve=False):
                    xr = xr_t.pop(t)
                    u = pc_sb.tile([128, C], f32, name="u", tag="u")
                    for (j, w, pp) in pps:
                        nc.vector.tensor_add(u[:, j:j + w], pp[:, :w], xr[:, j:j + w])
                    stats = pc_st.tile([128, nsub, 6], f32, name="stats", tag="stats")
                    for s in range(nsub):
                        nc.vector.bn_stats(out=stats[:, s, :], in_=u[:, ts(s, NSTAT)])
                    mv = pc_st.tile([128, 2], f32, name="mv", tag="mv")
                    nc.vector.bn_aggr(out=mv[:], in_=stats[:])
                    rstd = pc_st.tile([128, 1], f32, name="rstd", tag="rstd")
                    nc.scalar.activation(rstd[:], mv[:, 1:2],
                                         mybir.ActivationFunctionType.Sqrt, bias=eps_t[:])
                    nc.vector.reciprocal(rstd[:], rstd[:])
                    nmr = pc_st.tile([128, 1], f32, name="nmr", tag="nmr")
                    nc.vector.tensor_scalar(out=nmr[:], in0=mv[:, 0:1],
                                            scalar1=rstd[:], scalar2=-1.0,
                                            op0=mybir.AluOpType.mult,
                                            op1=mybir.AluOpType.mult)
                    of = pc_sb.tile([128, C], f32, name="of", tag="u")
                    # (u - mu)*rstd on ACT, then *gamma, +beta on GpSimd
                    nc.scalar.activation(of[:], u[:],
                                         mybir.ActivationFunctionType.Identity,
                                         scale=rstd[:], bias=nmr[:])
                    if affine_dve:
                        # final-flush tiles: gpsimd chains would serialize the
                        # tail; DVE is idle there
                        nc.vector.tensor_mul(of[:], of[:], gam_bc[:])
                        nc.vector.tensor_add(of[:], of[:], bet_bc[:])
                    else:
                        nc.gpsimd.tensor_mul(of[:], of[:], gam_bc[:])
                        nc.gpsimd.tensor_add(of[:], of[:], bet_bc[:])
                    nc.sync.dma_start(out_d.ap()[ts(t, 128), :], of[:])

                def proj_full(t):
                    proj_prefetch(t)
                    pps = [proj_chunk(t, 0, 512, pb_x1, "ps_x1"),
                           proj_chunk(t, 512, 256, pb_x1, "ps_x1")]
                    proj_finish(t, pps)

                pending_sm = None
                slot = 0
                proj_state = {}  # t -> list of pps
                for qb in range(TQ // BLK):
                    for h in range(H):
                        g, r = divmod(h * D, 128)
                        for tt in range(BLK):
                            t = qb * BLK + tt
                            q_s = qg[g][r:r + D, ts(t, 128)]
                            p_t = pb_p.tile([128, Nk], f16, name="p_t", tag="p_t")
                            nm_pack = pb_st.tile([128, NQS], f32, name="nm_pack", tag="nm_pack")
                            l_pack = pb_st.tile([128, NQS], f32, name="l_pack", tag="l_pack")
                            for j2 in range(NQS):
                                ps_s = pb_s.tile([128, JQ], f32, name="ps_s", tag="ps_s")
                                for jj in range(JQ // J):
                                    sl = slice(j2 * JQ + jj * J, j2 * JQ + (jj + 1) * J)
                                    nc.tensor.matmul(ps_s[:, ts(jj, J)], q_s,
                                                     kg[g][r:r + D, sl],
                                                     start=True, stop=True)
                                nc.vector.reduce_max(out=nm_pack[:, j2:j2 + 1], in_=ps_s[:],
                                                     axis=mybir.AxisListType.X, negate=True)
                                nc.scalar.activation(p_t[:, ts(j2, JQ)], ps_s[:],
                                                     mybir.ActivationFunctionType.Exp,
                                                     bias=nm_pack[:, j2:j2 + 1],
                                                     accum_out=l_pack[:, j2:j2 + 1])
                            if tt == 0:
                                pThb = pb_pth.tile([128, TK, BLK, 128], f16, name="pThb", tag="pThb")
                            # negm/e_p stay in-tile (depend only on this tile's
                            # maxes); the lagged tail runs next tile
                            negm = pb_st.tile([128, 1], f32, name="negm", tag="negm")
                            nc.vector.tensor_reduce(out=negm[:], in_=nm_pack[:],
                                                    axis=mybir.AxisListType.X,
                                                    op=mybir.AluOpType.min)
                            e_p = pb_st.tile([128, NQS], f32, name="e_p", tag="e_p")
                            nc.scalar.activation(e_p[:], nm_pack[:],
                                                 mybir.ActivationFunctionType.Exp,
                                                 scale=-1.0, bias=negm[:])
                            if pending_sm is not None:
                                softmax_tail(*pending_sm, on_act=(slot % 4 == 1))
                            pending_sm = (p_t, e_p, l_pack, pThb, tt)
                            av_step()
                            # deferred Phase-A group (qb0)
                            if deferred:
                                inject_deferred()
                            # qb0's proj+LN injected into qb1's early slots
                            if proj_state is not None and qb == 1 and 2 <= h <= 5:
                                pt = h - 2
                                if tt == 1:
                                    proj_prefetch(pt)
                                    proj_state[pt] = [proj_chunk(pt, 0, 512, pb_s, "ps_s")]
                                elif tt == 2:
                                    proj_state[pt].append(proj_chunk(pt, 512, 256, pb_s, "ps_s"))
                                    proj_finish(pt, proj_state.pop(pt))
                            slot += 1
                        av_queue.append([(g, r, h, qb, pThb, []), 0])
                    # qb end: flush last head's final tile tail + its AV
                    if pending_sm is not None:
                        softmax_tail(*pending_sm, on_act=False)
                        pending_sm = None
                    if qb == TQ // BLK - 1:
                        while av_queue:
                            av_step()
                        # pipelined tail: keep PE streaming pp chunks while the
                        # per-tile LN chains drain behind
                        t4 = qb * BLK
                        for tt in range(BLK):
                            proj_prefetch(t4 + tt)
                        pps = {}
                        pools = [(pb_x1, "ps_x1"), (pb_s, "ps_s")]
                        for tt in range(BLK):
                            pool, tg = pools[tt % 2]
                            pps[tt] = [proj_chunk(t4 + tt, 0, 512, pool, tg),
                                       proj_chunk(t4 + tt, 512, 256, pool, tg)]
                            if tt >= 1:
                                proj_finish(t4 + tt - 1, pps.pop(tt - 1), affine_dve=True)
                        proj_finish(t4 + BLK - 1, pps.pop(BLK - 1), affine_dve=True)
                        proj_state = None

    nc.compile()
    return nc


_CACHE = {}


def _get_program(cfg: Cfg):
    if cfg not in _CACHE:
        _CACHE[cfg] = build_program(cfg)
    return _CACHE[cfg]


def make_in_maps(x, qkv_w, proj_w, proj_b, ln_gamma, ln_beta, cfg: Cfg):
    """Host-side shard prep. Returns list of 8 in_maps."""
    C = cfg.C
    B = x.shape[0]
    wq_h = np.ascontiguousarray((qkv_w[0:C] * np.float32(cfg.D ** 0.5)).T.astype(np.float16))
    wk_h = np.ascontiguousarray(qkv_w[C:2 * C].T.astype(np.float16))
    wv_h = np.ascontiguousarray(qkv_w[2 * C:3 * C].T.astype(np.float16))
    wp_h = np.ascontiguousarray(proj_w.T.astype(np.float16))
    vecs = np.ascontiguousarray(np.stack([ln_gamma, ln_beta]).astype(np.float32))
    pb32 = proj_b.astype(np.float32)[None, :]
    in_maps = []
    for core in range(8):
        b, half = core // 2, core % 2
        b = min(b, B - 1)
        xb = np.asarray(x[b], dtype=np.float32)
        if half == 0:
            xkc = np.ascontiguousarray(xb)
        else:
            xkc = np.ascontiguousarray(np.concatenate([xb[cfg.Nq:], xb[:cfg.Nq]], axis=0))
        in_maps.append({"xh16": xkc.astype(np.float16),
                        "xq": np.ascontiguousarray(xkc[:cfg.Nq] + pb32),
                        "wq_h": wq_h, "wk_h": wk_h, "wv_h": wv_h,
                        "wp_h": wp_h, "vecs": vecs})
    return in_maps


def kernel(x, qkv_w, proj_w, proj_b, ln_gamma, ln_beta):
    from concourse.bass_utils import run_bass_kernel_spmd

    cfg = Cfg()
    nc = _get_program(cfg)
    x = np.asarray(x, dtype=np.float32)
    in_maps = make_in_maps(x, np.asarray(qkv_w, np.float32), np.asarray(proj_w, np.float32),
                           np.asarray(proj_b, np.float32), np.asarray(ln_gamma, np.float32),
                           np.asarray(ln_beta, np.float32), cfg)
    res = run_bass_kernel_spmd(nc, in_maps, core_ids=list(range(8)))
    B, N, C = x.shape
    out = np.empty((B, N, C), dtype=np.float32)
    for core in range(8):
        b, half = core // 2, core % 2
        out[b, half * cfg.Nq:(half + 1) * cfg.Nq] = res.results[core]["out"]
    return out
